# revision 4
# baseline (speedup 1.0000x reference)
"""Chamfer distance (squared-L2 NN, both directions) on 8 Trainium2 cores.

Sharding: 8 cores = 4 batches x 2 directions. Core 2b computes, for batch b,
min_m d^2(p1[n], p2[m]) for every n (p1 stationary); core 2b+1 the reverse
direction (p2 stationary). Host builds augmented K-row operands so a single
matmul produces squared distances directly in PSUM.

fp32 mode (K=5, exact but PE runs at 1/4 rate):
    lhsT rows: [|p|^2, -2px, -2py, -2pz, 1]
    rhs  rows: [1,     qx,   qy,   qz, |q|^2]
bf16x2 mode (K=13, hi/lo-split bf16 => full PE rate, ~2^-17 products):
    every fp32 value v is split v = vh + vl (both bf16); the distance
    s1 - 2<p,q> + s2 expands into 13 row pairs (dropping xl*yl terms).

Per stationary tile (128 points) the kernel runs 16 moving chunks of 512 as
4 PSUM groups of [128, 2048]; one vector tensor_reduce(min) per group writes
a per-point partial-min column, and one final segmented reduce produces the
per-point NN distance. Host averages and sums the two directions.
"""

import sys

sys.path.insert(0, "/opt/trn_rl_repo")

import numpy as np

B, N, M = 4, 8192, 8192
NCORES = 8
PTS = 8192          # stationary = moving = 8192 points per core
TS = 128            # stationary tile (partition dim)
CH = 512            # moving chunk (one PSUM bank)
GRP = 4             # chunks per PSUM group
NT = PTS // TS      # 64 stationary tiles
NG = PTS // (CH * GRP)  # 4 groups per stationary tile
GW = CH * GRP       # 2048 group width

MODE = "bf16x2"     # "f32" | "bf16x2"
KROWS = {"f32": 5, "bf16x2": 13}

_CACHE = {}


def _build_program(mode=MODE, repeats=1):
    from concourse import bacc, mybir, tile

    f32 = mybir.dt.float32
    mm_dt = f32 if mode == "f32" else mybir.dt.bfloat16
    mn = mybir.AluOpType.min
    K = KROWS[mode]

    nc = bacc.Bacc("TRN2", target_bir_lowering=False, debug=False,
                   num_devices=NCORES)
    sa_d = nc.dram_tensor("SA", [K, PTS], mm_dt, kind="ExternalInput")
    sm_d = nc.dram_tensor("SM", [K, PTS], mm_dt, kind="ExternalInput")
    out_d = nc.dram_tensor("MINS", [TS, NT], f32, kind="ExternalOutput")

    with tile.TileContext(nc) as tc:
        with (
            tc.tile_pool(name="inp", bufs=1) as inp,
            tc.tile_pool(name="acc", bufs=1) as acc,
            tc.tile_pool(name="psum", bufs=2, space="PSUM") as psum,
        ):
            sa = inp.tile([K, PTS], mm_dt)
            sm = inp.tile([K, PTS], mm_dt)
            nc.sync.dma_start(out=sa[:], in_=sa_d[:])
            nc.sync.dma_start(out=sm[:], in_=sm_d[:])

            d1g = acc.tile([TS, NT * NG], f32)
            d1 = acc.tile([TS, NT], f32)

            def main_pass(_iv=None):
                for t in range(NT):
                    for g in range(NG):
                        ps = psum.tile([TS, GW], f32, name="ps", tag="ps")
                        for j in range(GRP):
                            c = g * GRP + j
                            nc.tensor.matmul(
                                ps[:, j * CH:(j + 1) * CH],
                                lhsT=sa[:, t * TS:(t + 1) * TS],
                                rhs=sm[:, c * CH:(c + 1) * CH],
                                start=True, stop=True,
                            )
                        nc.vector.tensor_reduce(
                            out=d1g[:, t * NG + g:t * NG + g + 1],
                            in_=ps[:],
                            axis=mybir.AxisListType.X,
                            op=mn,
                        )

            if repeats == 1:
                main_pass()
            else:
                with tc.For_i(0, repeats, 1) as iv:
                    main_pass(iv)

            nc.vector.tensor_reduce(
                out=d1[:],
                in_=d1g[:].rearrange("p (t g) -> p t g", g=NG),
                axis=mybir.AxisListType.X,
                op=mn,
            )
            nc.sync.dma_start(out=out_d[:], in_=d1[:])

    nc.compile()
    return nc


def _bf16(x):
    import ml_dtypes
    return x.astype(ml_dtypes.bfloat16)


def _split(x):
    """fp32 -> (hi, lo) bf16 pair with hi + lo ~= x to ~2^-17."""
    hi = _bf16(x)
    lo = _bf16(x - hi.astype(np.float32))
    return hi, lo


def _aug_stationary(p, mode=MODE):
    s = np.sum(p.astype(np.float32) ** 2, axis=1, dtype=np.float32)
    n = p.shape[0]
    if mode == "f32":
        out = np.empty((5, n), np.float32)
        out[0] = s
        out[1:4] = -2.0 * p.T
        out[4] = 1.0
        return out
    xh, xl = _split(p.T.astype(np.float32))
    sh, sl = _split(s)
    import ml_dtypes
    out = np.empty((13, n), ml_dtypes.bfloat16)
    out[0] = sh
    out[1] = sl
    out[2:5] = _bf16(-2.0 * xh.astype(np.float32))
    out[5:8] = _bf16(-2.0 * xh.astype(np.float32))
    out[8:11] = _bf16(-2.0 * xl.astype(np.float32))
    out[11] = 1.0
    out[12] = 1.0
    return out


def _aug_moving(q, mode=MODE):
    s = np.sum(q.astype(np.float32) ** 2, axis=1, dtype=np.float32)
    n = q.shape[0]
    if mode == "f32":
        out = np.empty((5, n), np.float32)
        out[0] = 1.0
        out[1:4] = q.T
        out[4] = s
        return out
    yh, yl = _split(q.T.astype(np.float32))
    sh, sl = _split(s)
    import ml_dtypes
    out = np.empty((13, n), ml_dtypes.bfloat16)
    out[0] = 1.0
    out[1] = 1.0
    out[2:5] = yh
    out[5:8] = yl
    out[8:11] = yh
    out[11] = sh
    out[12] = sl
    return out


def kernel(p1, p2):
    from concourse.bass_utils import run_bass_kernel_spmd

    p1 = np.asarray(p1, np.float32)
    p2 = np.asarray(p2, np.float32)

    if "nc" not in _CACHE:
        _CACHE["nc"] = _build_program()
    nc = _CACHE["nc"]

    in_maps = []
    for core in range(NCORES):
        b, rev = divmod(core, 2)
        stat, mov = (p1[b], p2[b]) if rev == 0 else (p2[b], p1[b])
        in_maps.append({"SA": _aug_stationary(stat), "SM": _aug_moving(mov)})

    res = run_bass_kernel_spmd(nc, in_maps, core_ids=list(range(NCORES)))

    d1_all, d2_all = [], []
    for core in range(NCORES):
        mins = res.results[core]["MINS"]            # [128, 64]
        vals = np.maximum(mins.T.reshape(-1), 0.0)  # point index t*128+p
        (d1_all if core % 2 == 0 else d2_all).append(vals)

    out = np.float32(np.mean(np.stack(d1_all)) + np.mean(np.stack(d2_all)))
    return np.asarray(out, dtype=np.float32)
